# revision 34
# baseline (speedup 1.0000x reference)
"""Trainium2 Bass kernel for the ConcreteLayer training forward pass.

Computes out = x @ softmax((weight - ln(-ln((1-tiny)*uniform + tiny))) / T, axis=1)

Strategy (8 NeuronCores, 4x2 grid):
  - 4 batch groups x 2 out-column halves; core = 2*p + q.
  - Softmax numerators: Ln, Ln (scalar), w - m (vector), Exp (scalar, wide
    tiles, no accumulator); per-row sums via vector tensor_reduce.  The
    scalar queue is software-pipelined with a one-chunk skew so the
    cross-engine sub never stalls it.
  - Row sums are exchanged between column-half siblings with three small
    AllGathers (the CC stream warms up after the first op, ~6us each).
  - GEMM: bf16 lhsT (pre-transposed, host-cast x slice, fully resident
    in SBUF) x normalized bf16 numerators, f32 PSUM, all 8 banks.
  - Dummy matmuls gated on the first exchange trigger warm the PE clock
    (HAM) so the real GEMM starts at 2.4 GHz.
"""

import sys

import numpy as np

for _p in ("/opt/trn_rl_repo",):
    if _p not in sys.path:
        sys.path.insert(0, _p)

B, IN, OUT = 4096, 4096, 1024
GB, GO = 4, 2  # batch groups x out-half groups
BS = B // GB  # 1024 batch rows per core
OH = OUT // GO  # 512 out cols per core
P = 128
KT = IN // P  # 32 contraction tiles
KG = 4  # ktiles per softmax chunk
MBT = BS // P  # 8 output row tiles per core
NCORES = 8
GROUPS = [8, 12, 12]  # ktiles per row-sum exchange group
TINY = float(np.finfo(np.float32).tiny)

_PROGRAM = None
LAST_RESULT = None


def _pin_act_tables():
    """Steer the act-table-load pass to one set (has both Ln and Exp) so the
    compiler emits one ACT_TABLE_LOAD instead of reloading per tile."""
    import concourse.mybir as mybir
    from concourse import bacc, hw_specs

    orig = hw_specs.get_activation_tables.__wrapped__
    target = "natural_log_exp_and_others"
    strip = {
        mybir.ActivationFunctionType.Ln,
        mybir.ActivationFunctionType.Exp,
    }

    def pinned(arch):
        tables = orig(arch)
        if target not in tables:
            return tables
        return {
            name: (set(fns) if name == target else {f for f in fns if f not in strip})
            for name, fns in tables.items()
        }

    bacc.get_activation_tables = pinned


def _build_program():
    import concourse.bass as bass
    import concourse.mybir as mybir
    import concourse.tile as tile
    from concourse import bacc
    from contextlib import ExitStack

    _pin_act_tables()

    f32 = mybir.dt.float32
    bf16 = mybir.dt.bfloat16
    Ln = mybir.ActivationFunctionType.Ln
    Exp = mybir.ActivationFunctionType.Exp
    Alu = mybir.AluOpType

    nc = bacc.Bacc(
        "TRN2", target_bir_lowering=False, debug=False, num_devices=NCORES
    )

    xt_d = nc.dram_tensor("xt", [IN, BS], bf16, kind="ExternalInput")
    wh_d = nc.dram_tensor("wh", [IN, OH], bf16, kind="ExternalInput")
    uh_d = nc.dram_tensor("uh", [IN, OH], f32, kind="ExternalInput")
    t_d = nc.dram_tensor("tt", [1], f32, kind="ExternalInput")
    out_d = nc.dram_tensor("out", [BS, OH], f32, kind="ExternalOutput")

    replica_groups = [[0, 1], [2, 3], [4, 5], [6, 7]]
    NG = len(GROUPS)
    gbounds = []
    s = 0
    for gsz in GROUPS:
        gbounds.append((s, s + gsz))
        s += gsz
    assert s == KT
    NCH = KT // KG  # softmax chunks

    with tile.TileContext(nc) as tc, ExitStack() as ctx:
        dram = ctx.enter_context(tc.tile_pool(name="dram", bufs=1, space="DRAM"))
        singles = ctx.enter_context(tc.tile_pool(name="singles", bufs=1))
        chunks = ctx.enter_context(tc.tile_pool(name="chunks", bufs=3))
        outp = ctx.enter_context(tc.tile_pool(name="outp", bufs=4))
        psum = ctx.enter_context(tc.tile_pool(name="psum", bufs=1, space="PSUM"))

        # 1/T broadcast to all partitions.
        t_sb = singles.tile([P, 1], f32)
        t_ap = t_d.ap()
        nc.sync.dma_start(
            out=t_sb, in_=bass.AP(tensor=t_ap.tensor, offset=0, ap=[[0, P], [1, 1]])
        )
        invt = singles.tile([P, 1], f32)
        nc.vector.reciprocal(invt, t_sb)

        zero_t = singles.tile([P, 1], f32)
        nc.vector.memset(zero_t, 0.0)
        tiny_t = singles.tile([P, 1], f32)
        nc.vector.memset(tiny_t, TINY)

        # Resident tensors.
        xt_all = singles.tile([P, KT, BS], bf16)
        e_all = singles.tile([P, KT, OH], bf16)
        sums = singles.tile([P, KT], f32)
        r_all = singles.tile([P, KT], f32)

        cc_in = [
            dram.tile([P, gsz], f32, name=f"cc_in{g}", tag=f"cc_in{g}")
            for g, gsz in enumerate(GROUPS)
        ]
        cc_out = [
            dram.tile([2, P, gsz], f32, name=f"cc_out{g}", tag=f"cc_out{g}")
            for g, gsz in enumerate(GROUPS)
        ]

        ps_tiles = [
            psum.tile([P, OH], f32, tag=f"ps{mb}", name=f"ps{mb}")
            for mb in range(MBT)
        ]

        dumm = singles.tile([P, OH], bf16)

        def chunk_front(kb):
            """DMA + the two Ln passes + the sub for chunk kb."""
            base = kb * KG * P
            u_t = chunks.tile([P, KG, OH], f32, tag="u", name="u_t")
            w_t = chunks.tile([P, KG, OH], bf16, tag="w", name="w_t")
            u_src = uh_d[base : base + KG * P, :].rearrange("(g p) c -> p g c", p=P)
            w_src = wh_d[base : base + KG * P, :].rearrange("(g p) c -> p g c", p=P)
            nc.sync.dma_start(out=u_t, in_=u_src)
            nc.scalar.dma_start(out=w_t, in_=w_src)
            # v = ln((1 - tiny)*u + tiny)            (negative)
            nc.scalar.activation(u_t, u_t, Ln, bias=tiny_t[:], scale=1.0 - TINY)
            # m = ln(-v) = -gumbel
            nc.scalar.activation(u_t, u_t, Ln, bias=zero_t[:], scale=-1.0)
            # d = w - m = w + gumbel
            nc.vector.tensor_sub(u_t, w_t, u_t)
            return u_t

        def chunk_back(kb, u_t):
            """Wide Exp into e_all + per-ktile row sums for chunk kb."""
            nc.scalar.activation(
                e_all[:, kb * KG : (kb + 1) * KG, :],
                u_t,
                Exp,
                bias=zero_t[:],
                scale=invt[:],
            )
            for g in range(KG):
                ki = kb * KG + g
                nc.vector.tensor_reduce(
                    sums[:, ki : ki + 1],
                    e_all[:, ki, :],
                    mybir.AxisListType.X,
                    Alu.add,
                )

        def exchange(g):
            gs, ge = gbounds[g]
            nc.scalar.dma_start(out=cc_in[g], in_=sums[:, gs:ge])
            nc.gpsimd.collective_compute(
                "AllGather",
                Alu.bypass,
                replica_groups=replica_groups,
                ins=[cc_in[g].opt()],
                outs=[cc_out[g].opt()],
            )

        def readback(g):
            gs, ge = gbounds[g]
            gsz = ge - gs
            both = singles.tile([P, 2, gsz], f32, name=f"both{g}", tag=f"both{g}")
            nc.gpsimd.dma_start(
                out=both, in_=cc_out[g][:].rearrange("g p k -> p g k")
            )
            return both

        def finish(g, both):
            gs, ge = gbounds[g]
            gsz = ge - gs
            tot = singles.tile([P, gsz], f32, name=f"tot{g}", tag=f"tot{g}")
            nc.vector.tensor_add(tot, both[:, 0, :], both[:, 1, :])
            nc.vector.reciprocal(r_all[:, gs:ge], tot)
            for ki in range(gs, ge):
                nc.vector.tensor_scalar_mul(
                    e_all[:, ki, :], e_all[:, ki, :], r_all[:, ki : ki + 1]
                )
            for ki in range(gs, ge):
                for mb in range(MBT):
                    nc.tensor.matmul(
                        ps_tiles[mb][:],
                        lhsT=xt_all[:, ki, mb * P : (mb + 1) * P],
                        rhs=e_all[:, ki, :],
                        start=(ki == 0),
                        stop=(ki == KT - 1),
                    )

        # Software-pipelined emission: chunk kb's Exp+sums are emitted after
        # chunk kb+1's Ln passes, so the scalar FIFO never waits on the
        # cross-engine sub.  Exchanges fire as soon as their group's sums
        # are complete; finish(g) (normalize + GEMM) is emitted after the
        # next group's trigger so production is never queued behind a
        # CC-dependent op.
        XG = 2
        done_k = 0  # ktiles whose sums are emitted
        next_g = 0  # next exchange group to stage
        pend_fin = []  # groups with exchange fired, finish not yet emitted

        def maybe_exchange():
            nonlocal next_g
            while next_g < NG and gbounds[next_g][1] <= done_k:
                exchange(next_g)
                pend_fin.append(next_g)
                next_g += 1

        prev = None
        for kb in range(NCH):
            u_t = chunk_front(kb)
            if kb == 2:
                # xt prefetch on the gpsimd queue, gated behind the first
                # chunk's row sum so the early u/w loads get the DMA
                # engines to themselves (queue position alone would not
                # delay the transfers).
                xt_gate = singles.tile([P, 1], f32)
                nc.gpsimd.tensor_copy(xt_gate, sums[:, 0:1])
                for xb in range(KT // XG):
                    base = xb * XG * P
                    src = xt_d[base : base + XG * P, :].rearrange(
                        "(g p) b -> p g b", p=P
                    )
                    nc.gpsimd.dma_start(
                        out=xt_all[:, xb * XG : (xb + 1) * XG, :], in_=src
                    )
            if prev is not None:
                chunk_back(*prev)
                done_k = prev[0] * KG + KG
                maybe_exchange()
            prev = (kb, u_t)
        chunk_back(*prev)
        done_k = KT
        maybe_exchange()
        boths = [readback(g) for g in pend_fin]
        for g, both in zip(pend_fin, boths):
            finish(g, both)

        # Drain PSUM in column halves for finer store overlap.
        for mb in range(MBT):
            for h in range(2):
                o_t = outp.tile([P, OH // 2], f32, tag="o")
                nc.vector.tensor_copy(
                    o_t, ps_tiles[mb][:, h * (OH // 2) : (h + 1) * (OH // 2)]
                )
                nc.sync.dma_start(
                    out=out_d[
                        mb * P : (mb + 1) * P, h * (OH // 2) : (h + 1) * (OH // 2)
                    ],
                    in_=o_t,
                )

    nc.compile()
    return nc


def kernel(x, weight, uniform, T):
    global _PROGRAM, LAST_RESULT
    import ml_dtypes
    from concourse.bass_utils import run_bass_kernel_spmd

    if _PROGRAM is None:
        _PROGRAM = _build_program()
    nc = _PROGRAM

    bf = ml_dtypes.bfloat16
    x = np.asarray(x, dtype=np.float32)
    weight = np.asarray(weight, dtype=np.float32)
    uniform = np.ascontiguousarray(np.asarray(uniform, dtype=np.float32))
    T = np.ascontiguousarray(np.asarray(T, dtype=np.float32)).reshape([1])

    xt = np.ascontiguousarray(x.T.astype(bf))  # [IN, B] bf16
    wb = weight.astype(bf)
    in_maps = []
    for c in range(NCORES):
        p, q = c // GO, c % GO
        in_maps.append(
            {
                "xt": np.ascontiguousarray(xt[:, p * BS : (p + 1) * BS]),
                "wh": np.ascontiguousarray(wb[:, q * OH : (q + 1) * OH]),
                "uh": np.ascontiguousarray(uniform[:, q * OH : (q + 1) * OH]),
                "tt": T,
            }
        )

    res = run_bass_kernel_spmd(nc, in_maps, core_ids=list(range(NCORES)))
    LAST_RESULT = res

    out = np.empty((B, OUT), dtype=np.float32)
    for c in range(NCORES):
        p, q = c // GO, c % GO
        out[p * BS : (p + 1) * BS, q * OH : (q + 1) * OH] = res.results[c]["out"]
    return out
